# revision 27
# baseline (speedup 1.0000x reference)
"""MiMoV2 MoE gate (moe_routing) on 8 Trainium2 NeuronCores.

Strategy (v3):
  - Shard tokens (bsz*seq = 16384) across 8 cores, 2048 tokens each;
    replicate the [256, 4096] gate weight + bias.
  - Gating GEMM with W stationary and tokens moving (N=512), output
    [expert, token] in PSUM. Precision via fp16 main + ONE stacked
    fp8e4m3 DoubleRow correction pass:
      logits*2^17 = (x1*2^8)(W1*2^9)            [fp16, exact products]
                  + (dx*2^12)(W1*2^5)           [fp8 DR, chunk-paired]
                  + (x*2^-1)(dW*2^18)           [fp8 DR, chunk-paired]
    All three pieces share one PSUM accumulation; 2^-17 descale rides
    the psum->sbuf copy. Residual logit sigma ~1.3e-5.
  - v3 vs v2: HAM warmup shrunk from 64xN512 MMs (15.7us) to 8 fp32
    N=128 MMs on the identity tile (~3.4us, exactly the HAM window);
    x / W DRAM layouts flattened so every DMA is 128 fully-contiguous
    per-partition lines (descriptor-gen was 1.2us/DMA, now ~0.2);
    block 0 x1 delivered in 8ths so the real MM stream starts ~4us in;
    block 3 loaded contiguously (512 tokens) and MM'd as two 256-token
    halves reading SBUF slices.
  - Routing identical to v2: per-group top-2 via segmented reduce_max +
    match_replace; top-4 groups via max8 threshold; exact-passthrough
    masking; top-8 via max8 + max_index; weights via masked max8 over
    raw scores + 8x8 index-match permute.

Inputs (full):  hidden_states [4,4096,4096] f32, weight [256,4096] f32,
                e_score_correction_bias [256] f32
Output (full):  (topk_idx [16384,8] int32, topk_weight [16384,8] f32)
"""

import numpy as np
import ml_dtypes

import concourse.tile as tile
from concourse import bacc, mybir
from concourse.bass_utils import run_bass_kernel_spmd

# problem shape (hardcoded per contract)
T_FULL = 16384
H = 4096
E = 256
G = 8
GS = E // G           # 32
TOPK = 8
SCALING = 2.5

N_CORES = 8
T_CORE = T_FULL // N_CORES    # 2048
NCH = H // 128                # 32 contraction chunks
NQ = NCH // 2                 # 16 chunk-pairs for DoubleRow
TB = 512                      # token block (psum bank = 512 f32)
NB = T_CORE // TB             # 4 blocks
XF = NCH * TB                 # flat free size of one x block per partition
WF = NCH * 2 * 128            # flat free size of W per partition

SC_MAIN = 2.0 ** 17           # psum scale
S_X1 = 2.0 ** 8               # x1 pre-scale (x1*W1 -> 2^17)
S_W1 = 2.0 ** 9
S_DX = 2.0 ** 12              # dx pre-scale (dx*W1 -> 2^17)
S_W1_8 = 2.0 ** 5
S_XC = 2.0 ** -1              # coarse-x pre-scale (x*dW -> 2^17)
S_DW = 2.0 ** 18

_BUILT = None


def _build():
    f32 = mybir.dt.float32
    f16 = mybir.dt.float16
    f8 = mybir.dt.float8e4
    u32 = mybir.dt.uint32
    AF = mybir.ActivationFunctionType
    OP = mybir.AluOpType
    AX = mybir.AxisListType
    DR = mybir.MatmulPerfMode.DoubleRow

    nc = bacc.Bacc("TRN2", target_bir_lowering=False, debug=False)

    # x arrays: flat per-partition layout, elem (b, p, c*TB+t) = x[c*128+p,
    # b*TB+t]; every DMA slice below is contiguous per partition.
    x1 = nc.dram_tensor("x1", [NB, 128, XF], f16, kind="ExternalInput").ap()
    dx8 = nc.dram_tensor("dx8", [NB, 128, XF], f8, kind="ExternalInput").ap()
    xc8 = nc.dram_tensor("xc8", [NB, 128, XF], f8, kind="ExternalInput").ap()
    # W arrays: flat [128, chunk*ehalf*128e]
    w1 = nc.dram_tensor("w1", [128, WF], f16, kind="ExternalInput").ap()
    w18 = nc.dram_tensor("w18", [128, WF], f8, kind="ExternalInput").ap()
    dw8 = nc.dram_tensor("dw8", [128, WF], f8, kind="ExternalInput").ap()
    bias_rep = nc.dram_tensor("bias_rep", [128, E], f32, kind="ExternalInput").ap()
    id_in = nc.dram_tensor("id_in", [128, 128], f32, kind="ExternalInput").ap()

    idx_out = nc.dram_tensor("idx_out", [T_CORE, TOPK], u32, kind="ExternalOutput").ap()
    w_out = nc.dram_tensor("w_out", [T_CORE, TOPK], f32, kind="ExternalOutput").ap()

    with tile.TileContext(nc) as tc:
        with tc.tile_pool(name="const", bufs=1) as cpool, \
             tc.tile_pool(name="xin", bufs=1) as xpool, \
             tc.tile_pool(name="comb", bufs=2) as kpool, \
             tc.tile_pool(name="mid", bufs=2) as mpool, \
             tc.tile_pool(name="small", bufs=2) as spool, \
             tc.tile_pool(name="pacc", bufs=2, space="PSUM") as papool, \
             tc.tile_pool(name="ptr", bufs=3, space="PSUM") as ptpool:

            # constants. Scalar's queue starts with ~6us of engine init
            # (iram + act tables), so the first W quarter goes on the
            # otherwise-idle Vector queue to land by ~5us; the rest stream
            # on Scalar. IDT/BR are only needed ~35us in.
            W1t = cpool.tile([128, WF], f16, tag="W1t")
            W18t = cpool.tile([128, WF], f8, tag="W18t")
            dW8t = cpool.tile([128, WF], f8, tag="dW8t")
            BR = cpool.tile([128, E], f32, tag="BR")
            IDT = cpool.tile([128, 128], f32, tag="IDT")
            # w1 quarters first on Scalar's ring (main MMs consume W1 first);
            # fp8 correction weights next (needed only when the DR phase
            # starts); x rides Sync's ring in parallel.
            QW = WF // 4
            for q in range(4):
                sl = slice(q * QW, (q + 1) * QW)
                nc.scalar.dma_start(W1t[:, sl], w1[:, sl])
            nc.gpsimd.dma_start(W18t[:, 0:QW], w18[:, 0:QW])
            nc.gpsimd.dma_start(dW8t[:, 0:QW], dw8[:, 0:QW])
            for q in range(1, 4):
                sl = slice(q * QW, (q + 1) * QW)
                nc.scalar.dma_start(W18t[:, sl], w18[:, sl])
                nc.scalar.dma_start(dW8t[:, sl], dw8[:, sl])
            nc.scalar.dma_start(IDT[:], id_in)
            nc.scalar.dma_start(BR[:], bias_rep)

            # 4D views for matmul operands: [128, chunk, ehalf, 128e]
            W1v = W1t[:].rearrange("p (c h e) -> p c h e", c=NCH, h=2)
            W18v = W18t[:].rearrange("p (c h e) -> p c h e", c=NCH, h=2)
            dW8v = dW8t[:].rearrange("p (c h e) -> p c h e", c=NCH, h=2)

            # x tiles: flat [128, XF]; per-block double buffering by tag
            xt1 = {}
            xd8 = {}
            xc8t = {}
            xv1 = {}
            xvd = {}
            xvc = {}
            for par in range(2):
                xt1[par] = xpool.tile([128, XF], f16, tag=f"x1_{par}",
                                      name=f"xt1_{par}")
                xd8[par] = xpool.tile([128, XF], f8, tag=f"dx_{par}",
                                      name=f"xd8_{par}")
                xc8t[par] = xpool.tile([128, XF], f8, tag=f"xc_{par}",
                                       name=f"xc8t_{par}")
                xv1[par] = xt1[par][:].rearrange("p (c t) -> p c t", c=NCH)
                xvd[par] = xd8[par][:].rearrange("p (c t) -> p c t", c=NCH)
                xvc[par] = xc8t[par][:].rearrange("p (c t) -> p c t", c=NCH)

            def load_block(b):
                # everything on Sync's HWDGE ring (best bandwidth); x1 first
                # -- the per-block MM order is main-then-DR, so dx8/xc8 have
                # ~15us of slack after x1
                par = b % 2
                if b == 0:
                    # first two pieces small (2 chunks) so the MM stream
                    # starts as early as the ring can deliver
                    S16 = XF // 16
                    bounds = [0, S16, 2 * S16] + \
                        [(i + 1) * (XF // 8) for i in range(1, 8)]
                    for i in range(len(bounds) - 1):
                        sl = slice(bounds[i], bounds[i + 1])
                        nc.sync.dma_start(xt1[par][:, sl], x1[b][:, sl])
                    Q4 = XF // 4
                    for i in range(4):
                        sl = slice(i * Q4, (i + 1) * Q4)
                        nc.sync.dma_start(xd8[par][:, sl], dx8[b][:, sl])
                        # xc8 is consumed last (final DR phase) -- ship it on
                        # the GpSimd ring to unload Sync's early bulk
                        nc.gpsimd.dma_start(xc8t[par][:, sl], xc8[b][:, sl])
                else:
                    HF = XF // 2
                    for i in range(2):
                        sl = slice(i * HF, (i + 1) * HF)
                        nc.sync.dma_start(xt1[par][:, sl], x1[b][:, sl])
                    for i in range(2):
                        sl = slice(i * HF, (i + 1) * HF)
                        nc.sync.dma_start(xd8[par][:, sl], dx8[b][:, sl])
                        xcq = nc.gpsimd if b == 1 else nc.sync
                        xcq.dma_start(xc8t[par][:, sl], xc8[b][:, sl])

            load_block(0)
            load_block(1)

            # (dram_block, token_offset, tokens, out_token_base)
            BLOCKS = [(0, 0, TB, 0), (1, 0, TB, TB), (2, 0, TB, 2 * TB),
                      (3, 0, TB // 2, 3 * TB),
                      (3, TB // 2, TB // 2, 3 * TB + TB // 2)]
            for bi, (bsrc, boff, tbs, tbase) in enumerate(BLOCKS):
                par = bsrc % 2
                if bi == 4:
                    # scheduler-only fence: keep bi3's transposes/chain ahead
                    # of bi4's MM stream in queue order so the second-to-last
                    # routing chain overlaps the last MM phase instead of
                    # serializing after it
                    tc.no_sync_barrier()
                if bi in (1, 2):
                    load_block(bsrc + 1)   # prefetch next dram block
                tsl = slice(boff, boff + tbs)

                # main fp16 MMs first (consume x1/W1, which arrive first),
                # then the fp8 DoubleRow correction MMs (dx8/xc8 have the
                # whole main phase of slack). LDWEIGHTS stay hidden: fp16
                # FWL loads (~53ns) < 240ns stream, DR loads (~213ns) <
                # ~277ns DR stream.
                ps = {}
                for h in range(2):
                    ps[h] = papool.tile([128, TB], f32, tag=f"ps{h}",
                                        name=f"ps_{h}")
                    for g in range(NCH):
                        nc.tensor.matmul(ps[h][:, 0:tbs], W1v[:, g, h, :],
                                         xv1[par][:, g, tsl],
                                         start=(g == 0), stop=False)
                    for q in range(NQ):
                        nc.tensor.matmul(ps[h][:, 0:tbs],
                                         W18v[:, 2 * q:2 * q + 2, h, :],
                                         xvd[par][:, 2 * q:2 * q + 2, tsl],
                                         perf_mode=DR, start=False,
                                         stop=False)
                    for q in range(NQ):
                        nc.tensor.matmul(ps[h][:, 0:tbs],
                                         dW8v[:, 2 * q:2 * q + 2, h, :],
                                         xvc[par][:, 2 * q:2 * q + 2, tsl],
                                         perf_mode=DR, start=False,
                                         stop=(q == NQ - 1))

                # ---- routing. DVE ops cost ~150-200ns fixed each, so the
                # per-subtile chains are batched across the block's NS
                # subtiles wherever the op semantics allow (everything except
                # the per-row max8/find_index8/match_replace ops). ----
                NS = tbs // 128
                oq = nc.sync if bi >= 3 else nc.gpsimd
                # psum->sbuf descale copy on GPSIMD: scalar's FIFO otherwise
                # serializes [cb(k) copies] behind [sigmoids(k-1)], which lag
                # a full block and pile both final chains after the last MM
                cb = {}
                for h in range(2):
                    cb[h] = kpool.tile([128, TB], f32, tag=f"cb{h}",
                                       name=f"cb_{h}")
                    if bi >= 3:
                        # scalar's queue is shallow here (sigmoids only), so
                        # the final blocks' copies resolve promptly and the
                        # scheduler can slot their transposes right after
                        # their MMs -- bi3's chain then overlaps bi4's MMs
                        nc.scalar.activation(cb[h][:, 0:tbs], ps[h][:, 0:tbs],
                                             AF.Copy, scale=1.0 / SC_MAIN)
                    else:
                        nc.vector.tensor_scalar(cb[h][:, 0:tbs],
                                                ps[h][:, 0:tbs],
                                                1.0 / SC_MAIN, None,
                                                op0=OP.mult)
                s_raw = mpool.tile([128, NS, E], f32, tag="s_raw", name="s_raw")
                for g in range(NS):
                    pt = ptpool.tile([128, E], f32, tag="pt", name="pt")
                    for h in range(2):
                        nc.tensor.transpose(pt[:, h * 128:(h + 1) * 128],
                                            cb[h][:, g * 128:(g + 1) * 128],
                                            IDT[:])
                    nc.scalar.activation(s_raw[:, g, :], pt[:], AF.Sigmoid)
                s_choice = mpool.tile([128, NS, E], f32, tag="s_choice",
                                      name="s_choice")
                BR_b = BR[:].unsqueeze(1).broadcast_to([128, NS, E])
                nc.vector.tensor_tensor(s_choice[:], s_raw[:], BR_b, op=OP.add)
                sc4 = s_choice[:].rearrange("p n (g s) -> p n g s", g=G)
                m1 = spool.tile([128, NS, G], f32, tag="m1", name="m1")
                nc.vector.reduce_max(m1[:], sc4, axis=AX.X)
                repl = mpool.tile([128, NS, E], f32, tag="repl", name="repl")
                for g in range(NS):
                    nc.vector.match_replace(repl[:, g, :], m1[:, g, :],
                                            s_choice[:, g, :], -1e30)
                m2 = spool.tile([128, NS, G], f32, tag="m2", name="m2")
                nc.vector.reduce_max(
                    m2[:], repl[:].rearrange("p n (g s) -> p n g s", g=G),
                    axis=AX.X)
                gsum = spool.tile([128, NS, G], f32, tag="gsum", name="gsum")
                nc.vector.tensor_tensor(gsum[:], m1[:], m2[:], op=OP.add)
                gs8 = spool.tile([128, NS, 8], f32, tag="gs8", name="gs8")
                pen = spool.tile([128, NS, G], f32, tag="pen", name="pen")
                for g in range(NS):
                    nc.vector.max(gs8[:, g, :], gsum[:, g, :])
                for g in range(NS):
                    nc.vector.tensor_scalar(pen[:, g, :], gsum[:, g, :],
                                            gs8[:, g, 3:4],
                                            -1e30, op0=OP.is_lt, op1=OP.mult)
                # reuses the dead "repl" buffers (repl is consumed by m2)
                s_mask = mpool.tile([128, NS, E], f32, tag="repl",
                                    name="s_mask")
                pen_b = pen[:].unsqueeze(3).broadcast_to([128, NS, G, GS])
                nc.vector.tensor_tensor(
                    s_mask[:].rearrange("p n (g s) -> p n g s", g=G),
                    sc4, pen_b, op=OP.add)
                v8 = spool.tile([128, NS, 8], f32, tag="v8", name="v8")
                i8 = spool.tile([128, NS, 8], u32, tag="i8", name="i8")
                for g in range(NS):
                    nc.vector.max(v8[:, g, :], s_mask[:, g, :])
                for g in range(NS):
                    nc.vector.max_index(i8[:, g, :], v8[:, g, :],
                                        s_mask[:, g, :])
                    tok0 = tbase + g * 128
                    oq.dma_start(idx_out[tok0:tok0 + 128, :], i8[:, g, :])
                r_sel = mpool.tile([128, NS, E], f32, tag="repl", name="r_sel")
                for g in range(NS):
                    nc.vector.scalar_tensor_tensor(
                        r_sel[:, g, :], in0=s_mask[:, g, :],
                        scalar=v8[:, g, 7:8],
                        in1=s_raw[:, g, :], op0=OP.is_ge, op1=OP.mult)
                w8d = spool.tile([128, NS, 8], f32, tag="w8d", name="w8d")
                ri8 = spool.tile([128, NS, 8], u32, tag="ri8", name="ri8")
                for g in range(NS):
                    nc.vector.max(w8d[:, g, :], r_sel[:, g, :])
                for g in range(NS):
                    nc.vector.max_index(ri8[:, g, :], w8d[:, g, :],
                                        r_sel[:, g, :])
                eq64 = spool.tile([128, NS, 8, 8], f32, tag="eq64", name="eq64")
                i8_b = i8[:].unsqueeze(3).broadcast_to([128, NS, 8, 8])
                ri8_b = ri8[:].unsqueeze(2).broadcast_to([128, NS, 8, 8])
                nc.vector.tensor_tensor(eq64[:], i8_b, ri8_b, op=OP.is_equal)
                w64 = spool.tile([128, NS, 8, 8], f32, tag="w64", name="w64")
                w8d_b = w8d[:].unsqueeze(2).broadcast_to([128, NS, 8, 8])
                nc.vector.tensor_tensor(w64[:], eq64[:], w8d_b, op=OP.mult)
                w8p = spool.tile([128, NS, 8], f32, tag="w8p", name="w8p")
                nc.vector.reduce_sum(w8p[:], w64[:], axis=AX.X)
                sum8 = spool.tile([128, NS], f32, tag="sum8", name="sum8")
                nc.vector.reduce_sum(sum8[:], w8d[:], axis=AX.X)
                rcp = spool.tile([128, NS], f32, tag="rcp", name="rcp")
                nc.vector.reciprocal(rcp[:], sum8[:])
                rs = spool.tile([128, NS], f32, tag="rs", name="rs")
                nc.vector.tensor_scalar(rs[:], rcp[:], SCALING, None,
                                        op0=OP.mult)
                wf = spool.tile([128, NS, 8], f32, tag="wf", name="wf")
                rs_b = rs[:].unsqueeze(2).broadcast_to([128, NS, 8])
                nc.vector.tensor_tensor(wf[:], w8p[:], rs_b, op=OP.mult)
                for g in range(NS):
                    tok0 = tbase + g * 128
                    oq.dma_start(w_out[tok0:tok0 + 128, :], wf[:, g, :])

    nc.compile()
    return nc


def _get_built():
    global _BUILT
    if _BUILT is None:
        _BUILT = _build()
    return _BUILT


def _part(a, inner):
    # [H, inner] -> [128, NCH, inner] with element (p, c, i) = a[c*128+p, i]
    return np.ascontiguousarray(a.reshape(NCH, 128, inner).transpose(1, 0, 2))


def _prep_in_maps(hidden_states, weight, e_score_correction_bias):
    f8 = ml_dtypes.float8_e4m3
    x = np.asarray(hidden_states, dtype=np.float32).reshape(T_FULL, H)
    xT = np.ascontiguousarray(x.T)                      # [H, T]
    x1f = xT.astype(np.float16)
    dx = xT - x1f.astype(np.float32)

    x1s = (x1f.astype(np.float32) * S_X1).astype(np.float16)   # exact scale
    dx8f = (dx * S_DX).astype(f8)
    xc8f = (xT * S_XC).astype(f8)

    W = np.asarray(weight, dtype=np.float32)
    Wt = np.ascontiguousarray(W.T)                      # [H, E]
    W1f = Wt.astype(np.float16)
    dW = Wt - W1f.astype(np.float32)
    w1h = _part((W1f.astype(np.float32) * S_W1).astype(np.float16), E)
    w18h = _part((W1f.astype(np.float32) * S_W1_8).astype(f8), E)
    dw8h = _part((dW * S_DW).astype(f8), E)
    w1h = np.ascontiguousarray(w1h.reshape(128, WF))
    w18h = np.ascontiguousarray(w18h.reshape(128, WF))
    dw8h = np.ascontiguousarray(dw8h.reshape(128, WF))

    b = np.asarray(e_score_correction_bias, dtype=np.float32)
    bias_rep = np.ascontiguousarray(np.tile(b[None, :], (128, 1)))
    ident = np.eye(128, dtype=np.float32)

    def blocks(a):
        # [128, NCH, T_CORE] -> [NB, 128, NCH*TB]
        v = a.reshape(128, NCH, NB, TB)
        return np.ascontiguousarray(v.transpose(2, 0, 1, 3)).reshape(NB, 128, XF)

    in_maps = []
    for c in range(N_CORES):
        sl = slice(c * T_CORE, (c + 1) * T_CORE)
        in_maps.append({
            "x1": blocks(_part(x1s[:, sl], T_CORE)),
            "dx8": blocks(_part(dx8f[:, sl], T_CORE)),
            "xc8": blocks(_part(xc8f[:, sl], T_CORE)),
            "w1": w1h, "w18": w18h, "dw8": dw8h,
            "bias_rep": bias_rep, "id_in": ident,
        })
    return in_maps


def kernel(hidden_states: np.ndarray, weight: np.ndarray,
           e_score_correction_bias: np.ndarray):
    in_maps = _prep_in_maps(hidden_states, weight, e_score_correction_bias)
    nc = _get_built()
    res = run_bass_kernel_spmd(nc, in_maps, list(range(N_CORES)))

    idx = np.concatenate([r["idx_out"] for r in res.results], axis=0).astype(np.int32)
    w = np.concatenate([r["w_out"] for r in res.results], axis=0).astype(np.float32)
    return idx, w


# revision 31
# speedup vs baseline: 1.1024x; 1.1024x over previous
"""MiMoV2 MoE gate (moe_routing) on 8 Trainium2 NeuronCores.

Strategy (v3):
  - Shard tokens (bsz*seq = 16384) across 8 cores, 2048 tokens each;
    replicate the [256, 4096] gate weight + bias.
  - Gating GEMM with W stationary and tokens moving (N=512), output
    [expert, token] in PSUM. Precision via fp16 main + ONE stacked
    fp8e4m3 DoubleRow correction pass:
      logits*2^17 = (x1*2^8)(W1*2^9)            [fp16, exact products]
                  + (dx*2^12)(W1*2^5)           [fp8 DR, chunk-paired]
                  + (x*2^-1)(dW*2^18)           [fp8 DR, chunk-paired]
    All three pieces share one PSUM accumulation; 2^-17 descale rides
    the psum->sbuf copy. Residual logit sigma ~1.3e-5.
  - v3 vs v2: HAM warmup shrunk from 64xN512 MMs (15.7us) to 8 fp32
    N=128 MMs on the identity tile (~3.4us, exactly the HAM window);
    x / W DRAM layouts flattened so every DMA is 128 fully-contiguous
    per-partition lines (descriptor-gen was 1.2us/DMA, now ~0.2);
    block 0 x1 delivered in 8ths so the real MM stream starts ~4us in;
    block 3 loaded contiguously (512 tokens) and MM'd as two 256-token
    halves reading SBUF slices.
  - Routing identical to v2: per-group top-2 via segmented reduce_max +
    match_replace; top-4 groups via max8 threshold; exact-passthrough
    masking; top-8 via max8 + max_index; weights via masked max8 over
    raw scores + 8x8 index-match permute.

Inputs (full):  hidden_states [4,4096,4096] f32, weight [256,4096] f32,
                e_score_correction_bias [256] f32
Output (full):  (topk_idx [16384,8] int32, topk_weight [16384,8] f32)
"""

import numpy as np
import ml_dtypes

import concourse.tile as tile
from concourse import bacc, mybir
from concourse.bass_utils import run_bass_kernel_spmd

# problem shape (hardcoded per contract)
T_FULL = 16384
H = 4096
E = 256
G = 8
GS = E // G           # 32
TOPK = 8
SCALING = 2.5

N_CORES = 8
T_CORE = T_FULL // N_CORES    # 2048
NCH = H // 128                # 32 contraction chunks
NQ = NCH // 2                 # 16 chunk-pairs for DoubleRow
TB = 512                      # token block (psum bank = 512 f32)
NB = T_CORE // TB             # 4 blocks
XF = NCH * TB                 # flat free size of one x block per partition
WF = NCH * 2 * 128            # flat free size of W per partition

SC_MAIN = 2.0 ** 17           # psum scale
S_X1 = 2.0 ** 8               # x1 pre-scale (x1*W1 -> 2^17)
S_W1 = 2.0 ** 9
S_DX = 2.0 ** 12              # dx pre-scale (dx*W1 -> 2^17)
S_W1_8 = 2.0 ** 5
S_XC = 2.0 ** -1              # coarse-x pre-scale (x*dW -> 2^17)
S_DW = 2.0 ** 18

_BUILT = None


def _build():
    f32 = mybir.dt.float32
    f16 = mybir.dt.float16
    f8 = mybir.dt.float8e4
    u32 = mybir.dt.uint32
    AF = mybir.ActivationFunctionType
    OP = mybir.AluOpType
    AX = mybir.AxisListType
    DR = mybir.MatmulPerfMode.DoubleRow

    nc = bacc.Bacc("TRN2", target_bir_lowering=False, debug=False)

    # x arrays: flat per-partition layout, elem (b, p, c*TB+t) = x[c*128+p,
    # b*TB+t]; every DMA slice below is contiguous per partition.
    x1 = nc.dram_tensor("x1", [NB, 128, XF], f16, kind="ExternalInput").ap()
    dx8 = nc.dram_tensor("dx8", [NB, 128, XF], f8, kind="ExternalInput").ap()
    xc8 = nc.dram_tensor("xc8", [NB, 128, XF], f8, kind="ExternalInput").ap()
    # W arrays: flat [128, chunk*ehalf*128e]
    w1 = nc.dram_tensor("w1", [128, WF], f16, kind="ExternalInput").ap()
    w18 = nc.dram_tensor("w18", [128, WF], f8, kind="ExternalInput").ap()
    dw8 = nc.dram_tensor("dw8", [128, WF], f8, kind="ExternalInput").ap()
    bias_rep = nc.dram_tensor("bias_rep", [128, E], f32, kind="ExternalInput").ap()
    id_in = nc.dram_tensor("id_in", [128, 128], f32, kind="ExternalInput").ap()

    idx_out = nc.dram_tensor("idx_out", [T_CORE, TOPK], u32, kind="ExternalOutput").ap()
    w_out = nc.dram_tensor("w_out", [T_CORE, TOPK], f32, kind="ExternalOutput").ap()

    with tile.TileContext(nc) as tc:
        with tc.tile_pool(name="const", bufs=1) as cpool, \
             tc.tile_pool(name="xin", bufs=1) as xpool, \
             tc.tile_pool(name="comb", bufs=3) as kpool, \
             tc.tile_pool(name="mid", bufs=2) as mpool, \
             tc.tile_pool(name="small", bufs=2) as spool, \
             tc.tile_pool(name="pacc", bufs=2, space="PSUM") as papool, \
             tc.tile_pool(name="ptr", bufs=4, space="PSUM") as ptpool:

            # constants. Scalar's queue starts with ~6us of engine init
            # (iram + act tables), so the first W quarter goes on the
            # otherwise-idle Vector queue to land by ~5us; the rest stream
            # on Scalar. IDT/BR are only needed ~35us in.
            W1t = cpool.tile([128, WF], f16, tag="W1t")
            W18t = cpool.tile([128, WF], f8, tag="W18t")
            dW8t = cpool.tile([128, WF], f8, tag="dW8t")
            BR = cpool.tile([128, E], f32, tag="BR")
            IDT = cpool.tile([128, 128], f32, tag="IDT")
            # w1 quarters first on Scalar's ring (main MMs consume W1 first);
            # fp8 correction weights next (needed only when the DR phase
            # starts); x rides Sync's ring in parallel.
            QW = WF // 4
            for q in range(4):
                sl = slice(q * QW, (q + 1) * QW)
                nc.scalar.dma_start(W1t[:, sl], w1[:, sl])
            nc.gpsimd.dma_start(W18t[:, 0:QW], w18[:, 0:QW])
            nc.gpsimd.dma_start(dW8t[:, 0:QW], dw8[:, 0:QW])
            for q in range(1, 4):
                sl = slice(q * QW, (q + 1) * QW)
                nc.scalar.dma_start(W18t[:, sl], w18[:, sl])
                nc.scalar.dma_start(dW8t[:, sl], dw8[:, sl])
            nc.scalar.dma_start(IDT[:], id_in)
            nc.scalar.dma_start(BR[:], bias_rep)

            # 4D views for matmul operands: [128, chunk, ehalf, 128e]
            W1v = W1t[:].rearrange("p (c h e) -> p c h e", c=NCH, h=2)
            W18v = W18t[:].rearrange("p (c h e) -> p c h e", c=NCH, h=2)
            dW8v = dW8t[:].rearrange("p (c h e) -> p c h e", c=NCH, h=2)

            # x tiles: flat [128, XF]; per-block double buffering by tag
            xt1 = {}
            xd8 = {}
            xc8t = {}
            xv1 = {}
            xvd = {}
            xvc = {}
            for par in range(2):
                xt1[par] = xpool.tile([128, XF], f16, tag=f"x1_{par}",
                                      name=f"xt1_{par}")
                xd8[par] = xpool.tile([128, XF], f8, tag=f"dx_{par}",
                                      name=f"xd8_{par}")
                xc8t[par] = xpool.tile([128, XF], f8, tag=f"xc_{par}",
                                       name=f"xc8t_{par}")
                xv1[par] = xt1[par][:].rearrange("p (c t) -> p c t", c=NCH)
                xvd[par] = xd8[par][:].rearrange("p (c t) -> p c t", c=NCH)
                xvc[par] = xc8t[par][:].rearrange("p (c t) -> p c t", c=NCH)

            def load_block(b):
                # everything on Sync's HWDGE ring (best bandwidth); x1 first
                # -- the per-block MM order is main-then-DR, so dx8/xc8 have
                # ~15us of slack after x1
                par = b % 2
                if b == 0:
                    # first two pieces small (2 chunks) so the MM stream
                    # starts as early as the ring can deliver
                    S16 = XF // 16
                    bounds = [0, S16, 2 * S16] + \
                        [(i + 1) * (XF // 8) for i in range(1, 8)]
                    for i in range(len(bounds) - 1):
                        sl = slice(bounds[i], bounds[i + 1])
                        nc.sync.dma_start(xt1[par][:, sl], x1[b][:, sl])
                    Q4 = XF // 4
                    for i in range(4):
                        sl = slice(i * Q4, (i + 1) * Q4)
                        nc.sync.dma_start(xd8[par][:, sl], dx8[b][:, sl])
                        nc.sync.dma_start(xc8t[par][:, sl], xc8[b][:, sl])
                else:
                    HF = XF // 2
                    for i in range(2):
                        sl = slice(i * HF, (i + 1) * HF)
                        nc.sync.dma_start(xt1[par][:, sl], x1[b][:, sl])
                    for i in range(2):
                        sl = slice(i * HF, (i + 1) * HF)
                        nc.sync.dma_start(xd8[par][:, sl], dx8[b][:, sl])
                        nc.sync.dma_start(xc8t[par][:, sl], xc8[b][:, sl])

            load_block(0)
            load_block(1)

            # (dram_block, token_offset, tokens, out_token_base)
            BLOCKS = [(0, 0, TB, 0), (1, 0, TB, TB), (2, 0, TB, 2 * TB),
                      (3, 0, TB // 2, 3 * TB),
                      (3, TB // 2, TB // 2, 3 * TB + TB // 2)]
            for bi, (bsrc, boff, tbs, tbase) in enumerate(BLOCKS):
                par = bsrc % 2
                if bi == 4:
                    # scheduler-only fence: keep bi3's transposes/chain ahead
                    # of bi4's MM stream in queue order so the second-to-last
                    # routing chain overlaps the last MM phase instead of
                    # serializing after it
                    tc.no_sync_barrier()
                if bi in (1, 2):
                    load_block(bsrc + 1)   # prefetch next dram block
                tsl = slice(boff, boff + tbs)

                # main fp16 MMs first (consume x1/W1, which arrive first),
                # then the fp8 DoubleRow correction MMs (dx8/xc8 have the
                # whole main phase of slack). LDWEIGHTS stay hidden: fp16
                # FWL loads (~53ns) < 240ns stream, DR loads (~213ns) <
                # ~277ns DR stream.
                ps = {}
                for h in range(2):
                    ps[h] = papool.tile([128, TB], f32, tag=f"ps{h}",
                                        name=f"ps_{h}")
                    for g in range(NCH):
                        nc.tensor.matmul(ps[h][:, 0:tbs], W1v[:, g, h, :],
                                         xv1[par][:, g, tsl],
                                         start=(g == 0), stop=False)
                    for q in range(NQ):
                        nc.tensor.matmul(ps[h][:, 0:tbs],
                                         W18v[:, 2 * q:2 * q + 2, h, :],
                                         xvd[par][:, 2 * q:2 * q + 2, tsl],
                                         perf_mode=DR, start=False,
                                         stop=False)
                    for q in range(NQ):
                        nc.tensor.matmul(ps[h][:, 0:tbs],
                                         dW8v[:, 2 * q:2 * q + 2, h, :],
                                         xvc[par][:, 2 * q:2 * q + 2, tsl],
                                         perf_mode=DR, start=False,
                                         stop=(q == NQ - 1))

                # ---- routing. DVE ops cost ~150-200ns fixed each, so the
                # per-subtile chains are batched across the block's NS
                # subtiles wherever the op semantics allow (everything except
                # the per-row max8/find_index8/match_replace ops). ----
                NS = tbs // 128
                oq = nc.sync if bi >= 3 else nc.gpsimd
                # psum->sbuf descale copy on GPSIMD: scalar's FIFO otherwise
                # serializes [cb(k) copies] behind [sigmoids(k-1)], which lag
                # a full block and pile both final chains after the last MM
                cb = {}
                for h in range(2):
                    cb[h] = kpool.tile([128, TB], f32, tag=f"cb{h}",
                                       name=f"cb_{h}")
                    if bi >= 3:
                        # scalar's queue is shallow here (sigmoids only), so
                        # the final blocks' copies resolve promptly and the
                        # scheduler can slot their transposes right after
                        # their MMs -- bi3's chain then overlaps bi4's MMs
                        nc.scalar.activation(cb[h][:, 0:tbs], ps[h][:, 0:tbs],
                                             AF.Copy, scale=1.0 / SC_MAIN)
                    else:
                        nc.vector.tensor_scalar(cb[h][:, 0:tbs],
                                                ps[h][:, 0:tbs],
                                                1.0 / SC_MAIN, None,
                                                op0=OP.mult)
                s_raw = mpool.tile([128, NS, E], f32, tag="s_raw", name="s_raw")
                for g in range(NS):
                    pt = ptpool.tile([128, E], f32, tag="pt", name="pt")
                    for h in range(2):
                        nc.tensor.transpose(pt[:, h * 128:(h + 1) * 128],
                                            cb[h][:, g * 128:(g + 1) * 128],
                                            IDT[:])
                    nc.scalar.activation(s_raw[:, g, :], pt[:], AF.Sigmoid)
                s_choice = mpool.tile([128, NS, E], f32, tag="s_choice",
                                      name="s_choice")
                BR_b = BR[:].unsqueeze(1).broadcast_to([128, NS, E])
                nc.vector.tensor_tensor(s_choice[:], s_raw[:], BR_b, op=OP.add)
                sc4 = s_choice[:].rearrange("p n (g s) -> p n g s", g=G)
                m1 = spool.tile([128, NS, G], f32, tag="m1", name="m1")
                nc.vector.reduce_max(m1[:], sc4, axis=AX.X)
                repl = mpool.tile([128, NS, E], f32, tag="repl", name="repl")
                for g in range(NS):
                    nc.vector.match_replace(repl[:, g, :], m1[:, g, :],
                                            s_choice[:, g, :], -1e30)
                m2 = spool.tile([128, NS, G], f32, tag="m2", name="m2")
                nc.vector.reduce_max(
                    m2[:], repl[:].rearrange("p n (g s) -> p n g s", g=G),
                    axis=AX.X)
                gsum = spool.tile([128, NS, G], f32, tag="gsum", name="gsum")
                nc.vector.tensor_tensor(gsum[:], m1[:], m2[:], op=OP.add)
                gs8 = spool.tile([128, NS, 8], f32, tag="gs8", name="gs8")
                pen = spool.tile([128, NS, G], f32, tag="pen", name="pen")
                for g in range(NS):
                    nc.vector.max(gs8[:, g, :], gsum[:, g, :])
                for g in range(NS):
                    nc.vector.tensor_scalar(pen[:, g, :], gsum[:, g, :],
                                            gs8[:, g, 3:4],
                                            -1e30, op0=OP.is_lt, op1=OP.mult)
                # reuses the dead "repl" buffers (repl is consumed by m2)
                s_mask = mpool.tile([128, NS, E], f32, tag="repl",
                                    name="s_mask")
                pen_b = pen[:].unsqueeze(3).broadcast_to([128, NS, G, GS])
                nc.vector.tensor_tensor(
                    s_mask[:].rearrange("p n (g s) -> p n g s", g=G),
                    sc4, pen_b, op=OP.add)
                v8 = spool.tile([128, NS, 8], f32, tag="v8", name="v8")
                i8 = spool.tile([128, NS, 8], u32, tag="i8", name="i8")
                for g in range(NS):
                    nc.vector.max(v8[:, g, :], s_mask[:, g, :])
                for g in range(NS):
                    nc.vector.max_index(i8[:, g, :], v8[:, g, :],
                                        s_mask[:, g, :])
                    tok0 = tbase + g * 128
                    oq.dma_start(idx_out[tok0:tok0 + 128, :], i8[:, g, :])
                r_sel = mpool.tile([128, NS, E], f32, tag="repl", name="r_sel")
                for g in range(NS):
                    nc.vector.scalar_tensor_tensor(
                        r_sel[:, g, :], in0=s_mask[:, g, :],
                        scalar=v8[:, g, 7:8],
                        in1=s_raw[:, g, :], op0=OP.is_ge, op1=OP.mult)
                w8d = spool.tile([128, NS, 8], f32, tag="w8d", name="w8d")
                ri8 = spool.tile([128, NS, 8], u32, tag="ri8", name="ri8")
                for g in range(NS):
                    nc.vector.max(w8d[:, g, :], r_sel[:, g, :])
                for g in range(NS):
                    nc.vector.max_index(ri8[:, g, :], w8d[:, g, :],
                                        r_sel[:, g, :])
                eq64 = spool.tile([128, NS, 8, 8], f32, tag="eq64", name="eq64")
                i8_b = i8[:].unsqueeze(3).broadcast_to([128, NS, 8, 8])
                ri8_b = ri8[:].unsqueeze(2).broadcast_to([128, NS, 8, 8])
                nc.vector.tensor_tensor(eq64[:], i8_b, ri8_b, op=OP.is_equal)
                w64 = spool.tile([128, NS, 8, 8], f32, tag="w64", name="w64")
                w8d_b = w8d[:].unsqueeze(2).broadcast_to([128, NS, 8, 8])
                nc.vector.tensor_tensor(w64[:], eq64[:], w8d_b, op=OP.mult)
                w8p = spool.tile([128, NS, 8], f32, tag="w8p", name="w8p")
                nc.vector.reduce_sum(w8p[:], w64[:], axis=AX.X)
                sum8 = spool.tile([128, NS], f32, tag="sum8", name="sum8")
                nc.vector.reduce_sum(sum8[:], w8d[:], axis=AX.X)
                rcp = spool.tile([128, NS], f32, tag="rcp", name="rcp")
                nc.vector.reciprocal(rcp[:], sum8[:])
                rs = spool.tile([128, NS], f32, tag="rs", name="rs")
                nc.vector.tensor_scalar(rs[:], rcp[:], SCALING, None,
                                        op0=OP.mult)
                wf = spool.tile([128, NS, 8], f32, tag="wf", name="wf")
                rs_b = rs[:].unsqueeze(2).broadcast_to([128, NS, 8])
                nc.vector.tensor_tensor(wf[:], w8p[:], rs_b, op=OP.mult)
                for g in range(NS):
                    tok0 = tbase + g * 128
                    oq.dma_start(w_out[tok0:tok0 + 128, :], wf[:, g, :])

    nc.compile()
    return nc


def _get_built():
    global _BUILT
    if _BUILT is None:
        _BUILT = _build()
    return _BUILT


def _part(a, inner):
    # [H, inner] -> [128, NCH, inner] with element (p, c, i) = a[c*128+p, i]
    return np.ascontiguousarray(a.reshape(NCH, 128, inner).transpose(1, 0, 2))


def _prep_in_maps(hidden_states, weight, e_score_correction_bias):
    f8 = ml_dtypes.float8_e4m3
    x = np.asarray(hidden_states, dtype=np.float32).reshape(T_FULL, H)
    xT = np.ascontiguousarray(x.T)                      # [H, T]
    x1f = xT.astype(np.float16)
    dx = xT - x1f.astype(np.float32)

    x1s = (x1f.astype(np.float32) * S_X1).astype(np.float16)   # exact scale
    dx8f = (dx * S_DX).astype(f8)
    xc8f = (xT * S_XC).astype(f8)

    W = np.asarray(weight, dtype=np.float32)
    Wt = np.ascontiguousarray(W.T)                      # [H, E]
    W1f = Wt.astype(np.float16)
    dW = Wt - W1f.astype(np.float32)
    w1h = _part((W1f.astype(np.float32) * S_W1).astype(np.float16), E)
    w18h = _part((W1f.astype(np.float32) * S_W1_8).astype(f8), E)
    dw8h = _part((dW * S_DW).astype(f8), E)
    w1h = np.ascontiguousarray(w1h.reshape(128, WF))
    w18h = np.ascontiguousarray(w18h.reshape(128, WF))
    dw8h = np.ascontiguousarray(dw8h.reshape(128, WF))

    b = np.asarray(e_score_correction_bias, dtype=np.float32)
    bias_rep = np.ascontiguousarray(np.tile(b[None, :], (128, 1)))
    ident = np.eye(128, dtype=np.float32)

    def blocks(a):
        # [128, NCH, T_CORE] -> [NB, 128, NCH*TB]
        v = a.reshape(128, NCH, NB, TB)
        return np.ascontiguousarray(v.transpose(2, 0, 1, 3)).reshape(NB, 128, XF)

    in_maps = []
    for c in range(N_CORES):
        sl = slice(c * T_CORE, (c + 1) * T_CORE)
        in_maps.append({
            "x1": blocks(_part(x1s[:, sl], T_CORE)),
            "dx8": blocks(_part(dx8f[:, sl], T_CORE)),
            "xc8": blocks(_part(xc8f[:, sl], T_CORE)),
            "w1": w1h, "w18": w18h, "dw8": dw8h,
            "bias_rep": bias_rep, "id_in": ident,
        })
    return in_maps


def kernel(hidden_states: np.ndarray, weight: np.ndarray,
           e_score_correction_bias: np.ndarray):
    in_maps = _prep_in_maps(hidden_states, weight, e_score_correction_bias)
    nc = _get_built()
    res = run_bass_kernel_spmd(nc, in_maps, list(range(N_CORES)))

    idx = np.concatenate([r["idx_out"] for r in res.results], axis=0).astype(np.int32)
    w = np.concatenate([r["w_out"] for r in res.results], axis=0).astype(np.float32)
    return idx, w
